# revision 8
# baseline (speedup 1.0000x reference)
"""Nearest-E8-lattice quantizer (CachedE8Quantizer) as a Bass/Tile kernel on 8 trn2 cores.

Input x: [8388608, 8] fp32. Output: nearest point of E8 = D8 u (D8 + 1/2).

Sharding: data-parallel over the points dim, 1/8 per core (no comms).

v3 algorithm — everything derives from ONE rounding r0 = round(x), d0 = x - r0:
  f1 = r0 + s/2 (s = sign(d0)), |d1| = 1/2 - |d0|, so with segment stats
  Sr = sum(r0), px = parity(#negative d0) (XOR-reduce of d0 sign bits),
  Sa = sum|d0|, ma = max|d0|, mina = min|d0|:
    p0 = parity(Sr); p1 = p0 XOR px
    D1 - D0 = (2 - Sa) + p1*2*mina - p0*(1 - 2*ma)  -> c0 = branch-0 chosen
  The D8 "flip the worst coordinate" step is performed by a SECOND magic
  rounding of a shifted input instead of an argmax/onehot/select chain:
    G/2 = c0*p0*(1/2 - ma + eps) - c1*p1*(mina + eps);  H = c1*1/2
    y = round(x ± G/2 - H) + H     (± = sign(d0), applied by XOR on sign bit)
  eps fuzz only affects near-ties of the argmax (~1e-6 of points).

Per-tile big ops: 1 ts round, 1 tt delta, 5 segmented reduces, 1 bit-stt,
1 tt add, 1 stt, 2 ACT rounds, 1 stt (+ ~22 small per-point ops), vs the
baseline's 16 DVE passes + PE matmuls. Output DMA'd as bf16 (exact for E8).
"""

import numpy as np

from concourse import bacc
import concourse.mybir as mybir
from concourse.alu_op_type import AluOpType as op
from concourse.bass_utils import run_bass_kernel_spmd
from concourse.tile import TileContext

N_POINTS = 8388608
N_CORES = 8
SHARD = N_POINTS // N_CORES  # 1048576 points per core

MAGIC = 12582912.0  # 1.5 * 2**23: (x + MAGIC) - MAGIC == round-half-even(x)
EPS = float(2.0 ** -20)
F32 = mybir.dt.float32
BF16 = mybir.dt.bfloat16
U32 = mybir.dt.uint32
X = mybir.AxisListType.X
CP = mybir.ActivationFunctionType.Copy

# engine per op: "v" = DVE, "g" = GPSIMD(Pool), "a" = ACT (1-input only)
# scalar_tensor_tensor (stt) ops are DVE-only (walrus rejects them on Pool)
ENGINES = {
    "r0": "v",
    "d0": "g",
    "g": "v",   # bit-stt: DVE only
    "v": "g",
    "u": "v",   # stt: DVE only
    "w": "a",   # "a" = 2 ACT copies with bias; "v"/"g" = 1 ts
    "y": "v",   # stt: DVE only
    "out_bf16": True,
    "tf": 256,
    "bufs": 4,
}


def _imm_u32(v):
    return mybir.ImmediateValue(dtype=U32, value=v)


def _ts_u32(eng, out, in0, s0, s1, op0, op1):
    """tensor_scalar with uint32 immediates (bit ops)."""
    return eng.add_instruction(
        mybir.InstTensorScalarPtr(
            name=eng.bass.get_next_instruction_name(),
            op0=op0,
            op1=op1,
            ins=[eng.lower_ap(in0), _imm_u32(s0), _imm_u32(s1)],
            outs=[eng.lower_ap(out)],
        )
    )


def _stt_u32(eng, out, in0, scalar_int, in1, op0, op1):
    """scalar_tensor_tensor with a uint32 immediate: out = (in0 op0 imm) op1 in1."""
    return eng.add_instruction(
        mybir.InstTensorScalarPtr(
            name=eng.bass.get_next_instruction_name(),
            is_scalar_tensor_tensor=True,
            op0=op0,
            op1=op1,
            ins=[eng.lower_ap(in0), _imm_u32(scalar_int), eng.lower_ap(in1)],
            outs=[eng.lower_ap(out)],
        )
    )


def _emit_front1(nc, pools, xd, t, tf):
    """DMA-in, round, delta for tile t."""
    E = lambda k: getattr(nc, {"v": "vector", "g": "gpsimd", "a": "scalar"}[ENGINES[k]])
    P = 128
    pts = P * tf
    FE = tf * 8
    stream, work, small = pools

    s = t * pts
    x_rows = xd[s : s + pts, :].rearrange("(p f) c -> p (f c)", p=P)
    xt = stream.tile([P, FE], F32, tag="xt")
    nc.sync.dma_start(out=xt[:], in_=x_rows)

    r0 = work.tile([P, FE], F32, tag="r0")
    d0 = work.tile([P, FE], F32, tag="d0")
    E("r0").tensor_scalar(r0[:], xt[:], MAGIC, MAGIC, op0=op.add, op1=op.subtract)
    E("d0").tensor_tensor(d0[:], xt[:], r0[:], op.subtract)
    return dict(t=t, xt=xt, r0=r0, d0=d0)


def _emit_front2(nc, pools, st, tf):
    """Segmented reduces + per-point smalls for tile st['t']."""
    P = 128
    FE = tf * 8
    stream, work, small = pools
    r0, d0 = st["r0"], st["d0"]

    d03 = d0[:].rearrange("p (t c) -> p t c", c=8)
    r03 = r0[:].rearrange("p (t c) -> p t c", c=8)
    d0u3 = d0[:].bitcast(U32).rearrange("p (t c) -> p t c", c=8)

    NS = 12
    arena = small.tile([P, NS * tf], F32, tag="arena")
    sl = lambda i: arena[:, i * tf : (i + 1) * tf]
    Sr, xr, Sa, ma, mina = sl(0), sl(1), sl(2), sl(3), sl(4)
    p0, p1, Gh, Hh = sl(5), sl(6), sl(7), sl(8)
    mm, cc1, tm2 = sl(9), sl(10), sl(11)
    xr_u = xr.bitcast(U32)

    nc.vector.tensor_reduce(Sr, r03, axis=X, op=op.add)
    nc.vector.tensor_reduce(xr_u, d0u3, axis=X, op=op.bitwise_xor)
    nc.vector.tensor_reduce(Sa, d03, axis=X, op=op.add, apply_absolute_value=True)
    nc.vector.tensor_reduce(ma, d03, axis=X, op=op.max, apply_absolute_value=True)
    nc.vector.tensor_reduce(mina, d03, axis=X, op=op.min, apply_absolute_value=True)

    A = nc.scalar     # affine smalls (1-input) on ACT
    T = nc.gpsimd     # plain tensor_tensor smalls on GPSIMD
    V = nc.vector

    # p0 = parity(Sr): pd = Sr - 2*round(Sr/2) in {-1,0,1}; p0 = |pd|
    A.activation(p0, Sr, CP, bias=MAGIC, scale=0.5)
    A.activation(p0, p0, CP, bias=-MAGIC)
    A.activation(p0, p0, CP, scale=-2.0)
    T.tensor_tensor(p0, Sr, p0, op.add)
    _ts_u32(V, p0.bitcast(U32), p0.bitcast(U32), 0x7FFFFFFF, 0x7FFFFFFF,
            op.bitwise_and, op.bitwise_and)

    # px = sign-parity; p1 = p0 XOR px = |p0 - px|
    _ts_u32(V, p1.bitcast(U32), xr_u, 0x80000000, 0x3F800000,
            op.bitwise_and, op.bitwise_or)
    A.activation(p1, p1, CP, bias=0.5, scale=-0.5)
    T.tensor_tensor(p1, p0, p1, op.subtract)
    _ts_u32(V, p1.bitcast(U32), p1.bitcast(U32), 0x7FFFFFFF, 0x7FFFFFFF,
            op.bitwise_and, op.bitwise_and)

    # margin m = (2 - Sa) + p1*2*mina - p0*(1 - 2*ma); c0 = (m >= 0)
    A.activation(mm, Sa, CP, bias=2.0, scale=-1.0)
    A.activation(tm2, mina, CP, scale=2.0)
    T.tensor_tensor(tm2, tm2, p1, op.mult)
    T.tensor_tensor(mm, mm, tm2, op.add)
    A.activation(tm2, ma, CP, bias=1.0, scale=-2.0)
    T.tensor_tensor(tm2, tm2, p0, op.mult)
    T.tensor_tensor(mm, mm, tm2, op.subtract)
    V.tensor_scalar(mm, mm, 0.0, None, op0=op.is_ge)

    # t0 = p0*(1/2 - ma + eps); t1 = -p1*(mina + eps); Gh = c1 ? t1 : t0
    A.activation(Gh, ma, CP, bias=0.5 + EPS, scale=-1.0)
    T.tensor_tensor(Gh, Gh, p0, op.mult)
    A.activation(Hh, mina, CP, bias=-EPS, scale=-1.0)
    T.tensor_tensor(Hh, Hh, p1, op.mult)
    A.activation(cc1, mm, CP, bias=1.0, scale=-1.0)
    V.copy_predicated(Gh, cc1.bitcast(U32), Hh)
    A.activation(Hh, cc1, CP, scale=0.5)

    st["Gh"] = Gh
    st["Hh"] = Hh
    return st


def _emit_back1(nc, pools, st, tf):
    """g (in-place into d0) and v = x + g for tile st['t']."""
    E = lambda k: getattr(nc, {"v": "vector", "g": "gpsimd", "a": "scalar"}[ENGINES[k]])
    P = 128
    FE = tf * 8
    stream, work, small = pools
    xt, d0, Gh = st["xt"], st["d0"], st["Gh"]

    d0u3 = d0[:].bitcast(U32).rearrange("p (t c) -> p t c", c=8)
    Gh_b = Gh.bitcast(U32).unsqueeze(2).broadcast_to([P, tf, 8])

    # g = (d0 & signmask) ^ Gh, written in place over d0
    _stt_u32(E("g"), d0u3, d0u3, 0x80000000, Gh_b, op.bitwise_and, op.bitwise_xor)
    vv = work.tile([P, FE], F32, tag="vv")
    E("v").tensor_tensor(vv[:], xt[:], d0[:], op.add)
    st["vv"] = vv
    return st


def _emit_back2(nc, pools, yd, st, tf):
    """u = v - H, w = round(u), y = w + H, DMA-out for tile st['t']."""
    E = lambda k: getattr(nc, {"v": "vector", "g": "gpsimd", "a": "scalar"}[ENGINES[k]])
    P = 128
    pts = P * tf
    FE = tf * 8
    stream, work, small = pools
    vv, Hh = st["vv"], st["Hh"]

    s = st["t"] * pts
    y_rows = yd[s : s + pts, :].rearrange("(p f) c -> p (f c)", p=P)
    H_b = Hh.unsqueeze(2).broadcast_to([P, tf, 8])

    vv3 = vv[:].rearrange("p (t c) -> p t c", c=8)
    E("u").scalar_tensor_tensor(vv3, H_b, -1.0, vv3, op0=op.mult, op1=op.add)
    if ENGINES["w"] == "a":
        nc.scalar.activation(vv[:], vv[:], CP, bias=MAGIC)
        nc.scalar.activation(vv[:], vv[:], CP, bias=-MAGIC)
    else:
        E("w").tensor_scalar(vv[:], vv[:], MAGIC, MAGIC, op0=op.add, op1=op.subtract)
    assert ENGINES["g"] == "v" and ENGINES["u"] == "v" and ENGINES["y"] == "v"
    ydt = BF16 if ENGINES["out_bf16"] else F32
    yt = stream.tile([P, FE], ydt, tag="yt")
    yt3 = yt[:].rearrange("p (t c) -> p t c", c=8)
    E("y").scalar_tensor_tensor(yt3, H_b, 1.0, vv3, op0=op.mult, op1=op.add)
    nc.sync.dma_start(out=y_rows, in_=yt[:])


def build_nc(shard=SHARD, tf=None, reps=1):
    P = 128
    tf = tf or ENGINES["tf"]
    pts = P * tf
    assert shard % pts == 0
    ntiles = shard // pts

    nc = bacc.Bacc("TRN2", target_bir_lowering=False, debug=False, num_devices=N_CORES)
    xd = nc.declare_dram_parameter("x", [shard, 8], F32, isOutput=False)
    ydt = BF16 if ENGINES["out_bf16"] else F32
    yd = nc.declare_dram_parameter("y", [shard, 8], ydt, isOutput=True)

    with TileContext(nc) as tc:
        with (
            tc.tile_pool(name="stream", bufs=ENGINES["bufs"]) as stream,
            tc.tile_pool(name="work", bufs=ENGINES["bufs"]) as work,
            tc.tile_pool(name="small", bufs=ENGINES["bufs"]) as small,
        ):
            for _ in range(reps):
                pools = (stream, work, small)
                stages = [None, None, None]  # [f1-done, f2-done, b1-done]
                for t in range(ntiles + 3):
                    nxt = _emit_front1(nc, pools, xd, t, tf) if t < ntiles else None
                    if stages[2] is not None:
                        _emit_back2(nc, pools, yd, stages[2], tf)
                    if stages[1] is not None:
                        stages[2] = _emit_back1(nc, pools, stages[1], tf)
                    else:
                        stages[2] = None
                    if stages[0] is not None:
                        stages[1] = _emit_front2(nc, pools, stages[0], tf)
                    else:
                        stages[1] = None
                    stages[0] = nxt
    nc.finalize()
    return nc


_BUILD_CACHE = {}


def _get_nc(shard, tf):
    key = (shard, tf)
    if key not in _BUILD_CACHE:
        _BUILD_CACHE[key] = build_nc(shard, tf)
    return _BUILD_CACHE[key]


def kernel(x: np.ndarray) -> np.ndarray:
    x = np.ascontiguousarray(x, dtype=np.float32)
    n = x.shape[0]
    shard = n // N_CORES
    tf = ENGINES["tf"]
    while shard % (128 * tf) != 0:
        tf //= 2
    nc = _get_nc(shard, tf)
    in_maps = [{"x": x[i * shard : (i + 1) * shard]} for i in range(N_CORES)]
    res = run_bass_kernel_spmd(nc, in_maps, list(range(N_CORES))).results
    out = np.concatenate([res[i]["y"] for i in range(N_CORES)], axis=0)
    return np.ascontiguousarray(out.astype(np.float32))
